# revision 43
# baseline (speedup 1.0000x reference)
"""GQA attention block (B=2, S=2048, D=1024, 16 q-heads / 4 kv-heads, RoPE,
softmax(QK^T/sqrt(D)) V, output projection) on 8 Trainium2 NeuronCores.

Sharding: core c = b*4 + g handles batch b and kv-group g (q-heads 4g..4g+3).
Each core computes its 4 heads' attention plus the corresponding 256 rows of
Wo, producing a partial (D, S) output; the host sums the 4 partials per batch.

v2 design notes (vs baseline):
  - K and V projections fused into one K=128 matmul (wkv = [Wv | Wk]).
  - Scores matmuls for the two heads of a pair are row-tiled (K=64 at
    tile_position (0,0) and (64,0)) and issued back-to-back so they run
    concurrently in the PE array.
  - The scalar engine is reserved exclusively for exp (the ~147us floor);
    all PSUM evacuation copies run on DVE.
  - Softmax denominator via ones-augmented V (PSUM row 64); normalize chain:
    DVE reciprocal on the denom row -> K=1 ones matmul broadcast -> DVE mul.
  - ctx stored pair-packed (head 2p at partitions 0-63, 2p+1 at 64-127 via
    SBUF->SBUF DMA) so the Wo projection runs as K=128 matmuls.
  - Output written bf16 (host accumulates in f32).
"""

import sys
if "/opt/trn_rl_repo" not in sys.path:
    sys.path.insert(0, "/opt/trn_rl_repo")

import numpy as np
import ml_dtypes

B, S, D = 2, 2048, 1024
H, G, HD = 16, 4, 64
NCORES = 8
QC = 512          # projection chunk (matmul free dim)
NQC = S // QC     # 4
NKT = S // 128    # 16 k-token tiles
QB = 1024         # attention q block
THETA = 10000.0

_compiled = None


def _build_program():
    import concourse.bass as bass
    import concourse.tile as tile
    import concourse.mybir as mybir
    from concourse import bacc
    from contextlib import ExitStack

    bf16 = mybir.dt.bfloat16
    f32 = mybir.dt.float32
    EXP = mybir.ActivationFunctionType.Exp

    nc = bacc.Bacc("TRN2", target_bir_lowering=False, debug=False,
                   num_devices=NCORES)

    def din(name, shape, dt=bf16):
        return nc.dram_tensor(name, shape, dt, kind="ExternalInput").ap()

    # All non-x inputs are pre-packed on the host into a few wide [128, N]
    # tensors: every dma_start costs ~650ns of SyncE issue time, so fewer,
    # larger transfers get x flowing (and phase B started) much earlier.
    xT = din("xT", [D, S])
    wkvr = din("wkvr", [128, 1024])    # 8 col-blocks: [Wv_g | Wk_g] k-tiles
    wqr = din("wqr", [128, 2048])      # 8 col-blocks: Wq k-tiles (256 wide)
    qtab = din("qtab", [128, 4 * S])   # [cq0 | cq1 | sq0 | sq1]
    cksk = din("cksk", [HD, 2 * S])    # [ck | sk]
    wor = din("wor", [128, 2048])      # [Wo rows 0-127 | rows 128-255]
    misc = din("misc", [128, 384])     # [perm | ident | osel]
    outT = nc.dram_tensor("outT", [D, S], bf16, kind="ExternalOutput").ap()

    with tile.TileContext(nc) as tc, ExitStack() as ctx:
        # ---------------- persistent SBUF tensors ----------------
        pers = ctx.enter_context(tc.tile_pool(name="pers", bufs=1))
        xt_s = [pers.tile([128, S], bf16, tag=f"xt{i}", name=f"xt{i}") for i in range(8)]
        wkv_a = pers.tile([128, 1024], bf16, tag="wkva", name="wkva")
        wq_a = pers.tile([128, 2048], bf16, tag="wqa", name="wqa")
        qtab_s = pers.tile([128, 4 * S], bf16, tag="qtab", name="qtab")
        ktab_s = pers.tile([128, 2 * S], bf16, tag="ktab", name="ktab")
        wo_a = pers.tile([128, 2048], bf16, tag="woa", name="woa")
        misc_s = pers.tile([128, 384], bf16, tag="misc", name="misc")
        # persistent denominator staging tiles for the row-selector
        # broadcast (rows other than 0/32 stay zero forever)
        denrp = [pers.tile([128, QB], bf16, tag=f"dn{i}", name=f"dn{i}")
                 for i in range(2)]

        # zero-padded per-head q tiles: qpad[pr][hx] has head (2pr+hx)'s
        # rotated q in rows 64*hx..64*hx+64 and ZEROS elsewhere, so scores
        # run as plain K=128 matmuls against kdup=[k;k] (no PE mode switch
        # between scores and PV -> no TensorE drains in the inner loop).
        qpad = [[pers.tile([128, S], bf16, tag=f"qp{i}{j}", name=f"qp{i}{j}")
                 for j in range(2)] for i in range(2)]
        kv_sb = pers.tile([128, S], bf16, tag="kv", name="kv")     # v rows 0-63, k rows 64-127
        kdup = pers.tile([128, S], bf16, tag="kdup", name="kdup")
        v_t = [pers.tile([128, HD + 1], bf16, tag=f"v{i}", name=f"v{i}") for i in range(NKT)]
        ctxn2 = [pers.tile([128, S], bf16, tag=f"cx{i}", name=f"cx{i}") for i in range(2)]

        # DMA order: weights + misc first (small), then the x tiles stream
        # in so pass 1 consumes them as they arrive; rope tables next; wo
        # last (phase D only).
        nc.sync.dma_start(wkv_a[:], wkvr[:])
        nc.sync.dma_start(wq_a[:], wqr[:])
        nc.sync.dma_start(misc_s[:], misc[:])
        for i in range(8):
            nc.sync.dma_start(xt_s[i][:], xT[128 * i:128 * (i + 1), :])
        nc.sync.dma_start(ktab_s[64:128, :], cksk[:])
        nc.sync.dma_start(qtab_s[:], qtab[:])
        nc.sync.dma_start(wo_a[:], wor[:])
        for i in range(2):
            nc.vector.memset(qpad[i][0][64:128, :], 0.0)
            nc.vector.memset(qpad[i][1][0:64, :], 0.0)
            nc.vector.memset(denrp[i][:], 0.0)

        # ---------------- phase B: projections + rope ----------------
        with tc.tile_pool(name="pj_ps", bufs=4, space="PSUM") as pj_ps, \
             tc.tile_pool(name="pj_swp", bufs=2, space="PSUM") as pj_swp, \
             tc.tile_pool(name="pj_aux", bufs=2, space="PSUM") as pj_aux, \
             tc.tile_pool(name="pj_sb", bufs=4) as pj_sb:

            def q_rope(mc, qc, q_ps_tile):
                sl = slice(qc * QC, (qc + 1) * QC)
                raw = pj_sb.tile([128, QC], bf16, tag="qraw", name="qraw")
                nc.vector.tensor_copy(raw[:], q_ps_tile[:])
                swp = pj_swp.tile([128, QC], f32, tag="swp", name="swp")
                nc.tensor.matmul(swp[:], misc_s[:, 0:128], raw[:],
                                 start=True, stop=True)
                t1 = pj_sb.tile([128, QC], bf16, tag="t1", name="t1")
                nc.vector.tensor_mul(
                    t1[:], raw[:],
                    qtab_s[:, S * mc + qc * QC:S * mc + (qc + 1) * QC])
                t2 = pj_sb.tile([128, QC], bf16, tag="t2", name="t2")
                nc.vector.tensor_mul(
                    t2[:], swp[:],
                    qtab_s[:, 2 * S + S * mc + qc * QC:
                           2 * S + S * mc + (qc + 1) * QC])
                nc.vector.tensor_add(qpad[mc][0][0:64, sl],
                                     t1[0:64, :], t2[0:64, :])
                nc.vector.tensor_add(qpad[mc][1][64:128, sl],
                                     t1[64:128, :], t2[64:128, :])

            def kv_rope(qc, kv_ps_tile):
                sl = slice(qc * QC, (qc + 1) * QC)
                nc.vector.tensor_copy(kv_sb[:, sl], kv_ps_tile[:])
                # k rope: raw k rows at partitions 64-127
                swp = pj_swp.tile([128, QC], f32, tag="swp", name="swp")
                nc.tensor.matmul(swp[64:128, :], misc_s[64:128, 64:128],
                                 kv_sb[64:128, sl], start=True, stop=True)
                t1 = pj_sb.tile([128, QC], bf16, tag="t1", name="t1")
                nc.vector.tensor_mul(t1[64:128, :], kv_sb[64:128, sl],
                                     ktab_s[64:128, sl])
                t2 = pj_sb.tile([128, QC], bf16, tag="t2", name="t2")
                nc.vector.tensor_mul(
                    t2[64:128, :], swp[64:128, :],
                    ktab_s[64:128, S + qc * QC:S + (qc + 1) * QC])
                nc.vector.tensor_add(kdup[64:128, sl], t1[64:128, :],
                                     t2[64:128, :])
                # duplicate k to partitions 0-63 for the T0 row tile
                nc.sync.dma_start(kdup[0:64, sl], kdup[64:128, sl])

            # ---- pass 1: fused V|K projection + Q head-pair 0, qc-halves,
            # MMs interleaved over arriving x tiles
            for half in range(2):
                qcs = (0, 1) if half == 0 else (2, 3)
                kv_ps = {qc: pj_ps.tile([128, QC], f32, tag="ps",
                                        name=f"kvps{qc}") for qc in qcs}
                q0_ps = {qc: pj_ps.tile([128, QC], f32, tag="ps",
                                        name=f"q0ps{qc}") for qc in qcs}
                for kt in range(8):
                    for qc in qcs:
                        nc.tensor.matmul(
                            kv_ps[qc][:], wkv_a[:, 128 * kt:128 * (kt + 1)],
                            xt_s[kt][:, qc * QC:(qc + 1) * QC],
                            start=(kt == 0), stop=(kt == 7))
                        nc.tensor.matmul(
                            q0_ps[qc][:], wq_a[:, 256 * kt:256 * kt + 128],
                            xt_s[kt][:, qc * QC:(qc + 1) * QC],
                            start=(kt == 0), stop=(kt == 7))
                for qc in qcs:
                    kv_rope(qc, kv_ps[qc])
                    q_rope(0, qc, q0_ps[qc])
            # v transposes: v_t[kt] = (v tile)^T with ones column
            for tt in range(NKT):
                tp = pj_aux.tile([128, QC], bf16, tag="aux", name="aux")
                nc.tensor.transpose(tp[:, :HD],
                                    kv_sb[:HD, 128 * tt:128 * (tt + 1)],
                                    misc_s[0:HD, 128:128 + HD])
                nc.vector.tensor_copy(v_t[tt][:, :HD], tp[:, :HD])
                nc.vector.memset(v_t[tt][:, HD:HD + 1], 1.0)

            # ---- pass 2: Q head-pair 1 (x already resident)
            for half in range(2):
                qcs = (0, 1) if half == 0 else (2, 3)
                q1_ps = {qc: pj_ps.tile([128, QC], f32, tag="ps",
                                        name=f"q1ps{qc}") for qc in qcs}
                for kt in range(8):
                    for qc in qcs:
                        nc.tensor.matmul(
                            q1_ps[qc][:], wq_a[:, 256 * kt + 128:256 * kt + 256],
                            xt_s[kt][:, qc * QC:(qc + 1) * QC],
                            start=(kt == 0), stop=(kt == 7))
                for qc in qcs:
                    q_rope(1, qc, q1_ps[qc])

        # ---------------- phase C: attention ----------------
        # Head pair pr = (2pr, 2pr+1): qrope[pr] partitions 0-63 / 64-127.
        # Scores row-tiled: T0 (kdup[0:64]) and T8 (kdup[64:128]) concurrent.
        INVSQ = 1.0 / 32.0  # 1/sqrt(D)
        with tc.tile_pool(name="at_s", bufs=2, space="PSUM") as at_s, \
             tc.tile_pool(name="at_c", bufs=2, space="PSUM") as at_c, \
             tc.tile_pool(name="at_p", bufs=4) as at_p, \
             tc.tile_pool(name="at_u", bufs=4) as at_u:

            def emit_lazy(d):
                """Deferred half of the softmax normalize: broadcast the
                denominator row via a K=128 row-selector matmul (same PE
                mode as everything else), reciprocal, scale, store."""
                pr_, qb_, ctxu2, b2 = d
                qsl = slice(qb_ * QB, (qb_ + 1) * QB)
                for hx in range(2):
                    bc = at_s.tile([128, QB], f32, tag="s", name="bc")
                    for h2 in range(2):
                        csl = slice(512 * h2, 512 * (h2 + 1))
                        nc.tensor.matmul(bc[0:HD, csl],
                                         misc_s[:, 256 + 64 * hx:256 + 64 * hx + HD],
                                         denrp[b2][:, csl],
                                         start=True, stop=True)
                    rcp = at_u.tile([HD, QB], f32, tag="rcp", name="rcp")
                    nc.vector.reciprocal_approx_fast(rcp[:], bc[0:HD, :])
                    if hx == 0:
                        nc.vector.tensor_mul(ctxn2[pr_][0:HD, qsl],
                                             ctxu2[hx][:], rcp[:])
                    else:
                        tmp = at_u.tile([HD, QB], bf16, tag="tmpb",
                                        name="tmpb")
                        nc.vector.tensor_mul(tmp[:], ctxu2[hx][:], rcp[:])
                        nc.sync.dma_start(ctxn2[pr_][HD:128, qsl], tmp[:])

            deferred = None
            blk = 0
            for qb in range(S // QB):
                q0 = qb * QB
                for pr in range(2):
                    ctxA = at_c.tile([HD + 1, QB], f32, tag="ctx", name="ctxA")
                    ctxB = at_c.tile([HD + 1, QB], f32, tag="ctx", name="ctxB")
                    # Software-pipelined: scores(kt)+exp(kt) emitted BEFORE
                    # PV(kt-1) so the PE queue never head-of-line blocks on
                    # an exp wait. Scores are K=128 matmuls against
                    # kdup=[k;k] with zero-padded per-head q: same PE mode
                    # as PV (no drains) and one shared stationary per kt.
                    # Chunk 0 of each head is issued TWICE (idempotent
                    # overwrite): keeps the PE ~100% busy so the HAM clock
                    # gate stays at full rate through the attention phase.
                    ps = [[None, None] for _ in range(NKT)]
                    ctxs = (ctxA, ctxB)
                    for kt in range(NKT + 1):
                        if kt < NKT:
                            ksl = slice(128 * kt, 128 * (kt + 1))
                            st = [at_s.tile([128, QB], f32, tag="s", name="s0"),
                                  at_s.tile([128, QB], f32, tag="s", name="s1")]
                            for hx in range(2):
                                for h2 in ((0, 0, 1) if hx == 0 else (0, 1)):
                                    csl = slice(512 * h2, 512 * (h2 + 1))
                                    qsl = slice(q0 + 512 * h2,
                                                q0 + 512 * (h2 + 1))
                                    nc.tensor.matmul(st[hx][:, csl],
                                                     kdup[:, ksl],
                                                     qpad[pr][hx][:, qsl],
                                                     start=True, stop=True)
                            for hx in range(2):
                                p = at_p.tile([128, QB], bf16, tag="pT",
                                              name=f"p{hx}")
                                nc.scalar.activation(p[:], st[hx][:], EXP,
                                                     scale=INVSQ)
                                ps[kt][hx] = p
                        if kt == 1 and deferred is not None:
                            # lazy half of the PREVIOUS block's normalize,
                            # emitted here so it fills the boundary instead
                            # of stalling it
                            emit_lazy(deferred)
                            deferred = None
                        if kt >= 1:
                            kp = kt - 1
                            for h2 in range(2):
                                csl = slice(512 * h2, 512 * (h2 + 1))
                                for hx in range(2):
                                    nc.tensor.matmul(ctxs[hx][:, csl],
                                                     v_t[kp][:],
                                                     ps[kp][hx][:, csl],
                                                     start=(kp == 0),
                                                     stop=(kp == NKT - 1))
                    # release half of the normalize: get the denominator row
                    # and the 64 ctx rows out of PSUM quickly, freeing both
                    # ctx banks for the next block.
                    b2 = blk % 2
                    ctxu2 = []
                    for hx, cx in ((0, ctxA), (1, ctxB)):
                        nc.scalar.copy(denrp[b2][32 * hx:32 * hx + 1, :],
                                       cx[HD:HD + 1, :])
                        cu = at_u.tile([HD, QB], bf16, tag="ctxu",
                                       name=f"ctxu{hx}")
                        nc.vector.tensor_copy(cu[:], cx[0:HD, :])
                        ctxu2.append(cu)
                    deferred = (pr, qb, ctxu2, b2)
                    blk += 1
            emit_lazy(deferred)

        # ---------------- phase D: output projection ----------------
        # qc-outer: the qb=0 columns go first (their normalize finished
        # mid-attention), so the pipe never waits on the final block.
        with tc.tile_pool(name="wo_ps", bufs=8, space="PSUM") as wo_ps, \
             tc.tile_pool(name="wo_sb", bufs=6) as wo_sb:
            for qc in range(NQC):
                sl = slice(qc * QC, (qc + 1) * QC)
                o_ps = [wo_ps.tile([128, QC], f32, tag="wops", name=f"wops{mc}")
                        for mc in range(8)]
                for pr in range(2):
                    for mc in range(8):
                        nc.tensor.matmul(
                            o_ps[mc][:], wo_a[:, 1024 * pr + 128 * mc:1024 * pr + 128 * (mc + 1)],
                            ctxn2[pr][:, sl], start=(pr == 0), stop=(pr == 1))
                for mc in range(8):
                    ob = wo_sb.tile([128, QC], bf16, tag="ob", name="ob")
                    if mc % 2 == 0:
                        nc.vector.tensor_copy(ob[:], o_ps[mc][:])
                    else:
                        nc.scalar.copy(ob[:], o_ps[mc][:])
                    nc.sync.dma_start(outT[128 * mc:128 * (mc + 1), sl], ob[:])

    nc.compile()
    return nc


def _host_inputs(x, Wq, Wk, Wv, Wo):
    """Build the 8 per-core input maps."""
    bf = ml_dtypes.bfloat16
    inv = 1.0 / (THETA ** (np.arange(0, D, 2, dtype=np.float64) / D))
    t = np.arange(S, dtype=np.float64)
    sgn256 = np.where(np.arange(256) % 2 == 0, -1.0, 1.0)
    sgn64 = sgn256[:HD]

    perm = np.zeros((128, 128), np.float32)
    idx = np.arange(128)
    perm[idx ^ 1, idx] = 1.0
    ident = np.eye(128, dtype=np.float32)
    osel = np.zeros((128, 128), np.float32)
    osel[0, 0:64] = 1.0      # head-A denominator row selector
    osel[32, 64:128] = 1.0   # head-B denominator row selector
    misc = np.concatenate([perm, ident, osel], axis=1)  # (128, 384)

    # k rope tables are core-independent
    angk = t[None, :] * inv[np.arange(HD) // 2][:, None]
    ck = np.cos(angk)
    sk = sgn64[:, None] * np.sin(angk)
    cksk = np.concatenate([ck, sk], axis=1).astype(bf)  # (64, 2S)

    in_maps = []
    for c in range(NCORES):
        b, g = divmod(c, G)
        fq = inv[128 * g + np.arange(256) // 2]
        angq = t[None, :] * fq[:, None]
        cq = np.cos(angq)
        sq = sgn256[:, None] * np.sin(angq)
        qtab = np.concatenate(
            [cq[0:128], cq[128:256], sq[0:128], sq[128:256]], axis=1)
        wkv = np.concatenate(
            [Wv[:, HD * g:HD * (g + 1)], Wk[:, HD * g:HD * (g + 1)]], axis=1)
        wkvr = np.concatenate(
            [wkv[128 * k:128 * (k + 1), :] for k in range(8)], axis=1)
        wq_g = Wq[:, 256 * g:256 * (g + 1)]
        wqr = np.concatenate(
            [wq_g[128 * k:128 * (k + 1), :] for k in range(8)], axis=1)
        wo_g = Wo[256 * g:256 * (g + 1), :]
        wor = np.concatenate([wo_g[0:128, :], wo_g[128:256, :]], axis=1)
        in_maps.append({
            "xT": np.ascontiguousarray(x[b].T).astype(bf),
            "wkvr": np.ascontiguousarray(wkvr).astype(bf),
            "wqr": np.ascontiguousarray(wqr).astype(bf),
            "qtab": np.ascontiguousarray(qtab).astype(bf),
            "cksk": cksk,
            "wor": np.ascontiguousarray(wor).astype(bf),
            "misc": misc.astype(bf),
        })
    return in_maps


def _run(in_maps, trace=False, tmpdir=None):
    global _compiled
    from concourse.bass_utils import run_bass_kernel_spmd
    if _compiled is None:
        _compiled = _build_program()
    return run_bass_kernel_spmd(_compiled, in_maps, list(range(NCORES)),
                                trace=trace, tmpdir=tmpdir)


def kernel(x, Wq, Wk, Wv, Wo, _trace=False, _tmpdir=None):
    x = np.asarray(x, np.float32)
    in_maps = _host_inputs(x, np.asarray(Wq, np.float32),
                           np.asarray(Wk, np.float32),
                           np.asarray(Wv, np.float32),
                           np.asarray(Wo, np.float32))
    res = _run(in_maps, trace=_trace, tmpdir=_tmpdir)
    out = np.zeros((B, S, D), np.float32)
    for c in range(NCORES):
        b = c // G
        out[b] += res.results[c]["outT"].T.astype(np.float32)
    kernel.last_results = res
    return out


# revision 45
# speedup vs baseline: 1.1813x; 1.1813x over previous
"""GQA attention block (B=2, S=2048, D=1024, 16 q-heads / 4 kv-heads, RoPE,
softmax(QK^T/sqrt(D)) V, output projection) on 8 Trainium2 NeuronCores.

Sharding: core c = b*4 + g handles batch b and kv-group g (q-heads 4g..4g+3).
Each core computes its 4 heads' attention plus the corresponding 256 rows of
Wo, producing a partial (D, S) output; the host sums the 4 partials per batch.

Design notes (vs the 412us baseline; measured ~253us):
  - The PE HAM clock gate is the dominant effect: at <~95% matmul density
    the PE gets stuck at 1.2 GHz. Everything in the attention loop is a
    single PE tiling mode (plain K=128 128x128 matmuls; no 64-row tiles,
    so no TensorE mode-switch drains) and the stream is kept dense with a
    deliberately duplicated (idempotent) score chunk per kt, which holds
    the HAM at K=8/8 (2.4 GHz) through the whole attention phase.
  - Scores use kdup=[k;k] with zero-padded per-head q tiles so both heads
    of a pair share one stationary; exp (ScalarE, the ~147us floor) is
    software-pipelined against PV (scores(kt)+exp(kt) emitted before
    PV(kt-1) so the PE FIFO never head-of-line blocks on an exp wait).
  - K and V projections fused into one K=128 matmul (wkv = [Wv | Wk]).
  - Softmax denominator rides the ones-augmented V (PSUM row 64); the
    normalize is split: a short release (ScalarE row copy + DVE ctx copy,
    frees both ctx PSUM banks fast) and a lazy half (K=128 row-selector
    broadcast matmul + reciprocal + scale) deferred into the next block.
  - All inputs are host-packed into a few wide [128, N] tensors: each
    dma_start costs ~650ns of SyncE issue time, so fewer transfers start
    phase B much earlier; x tiles stream while KV+Q projections consume.
  - ctx stored pair-packed (head 2p at partitions 0-63, 2p+1 at 64-127 via
    SBUF->SBUF DMA) so the Wo projection runs as K=128 matmuls; the qb=0
    half of Wo runs inside the attention scope to keep the PE warm while
    the final softmax normalize drains. Output written bf16 (host sums in
    f32).
"""

import sys
if "/opt/trn_rl_repo" not in sys.path:
    sys.path.insert(0, "/opt/trn_rl_repo")

import numpy as np
import ml_dtypes

B, S, D = 2, 2048, 1024
H, G, HD = 16, 4, 64
NCORES = 8
QC = 512          # projection chunk (matmul free dim)
NQC = S // QC     # 4
NKT = S // 128    # 16 k-token tiles
QB = 1024         # attention q block
THETA = 10000.0

_compiled = None


def _build_program():
    import concourse.bass as bass
    import concourse.tile as tile
    import concourse.mybir as mybir
    from concourse import bacc
    from contextlib import ExitStack

    bf16 = mybir.dt.bfloat16
    f32 = mybir.dt.float32
    EXP = mybir.ActivationFunctionType.Exp

    nc = bacc.Bacc("TRN2", target_bir_lowering=False, debug=False,
                   num_devices=NCORES)

    def din(name, shape, dt=bf16):
        return nc.dram_tensor(name, shape, dt, kind="ExternalInput").ap()

    # All non-x inputs are pre-packed on the host into a few wide [128, N]
    # tensors: every dma_start costs ~650ns of SyncE issue time, so fewer,
    # larger transfers get x flowing (and phase B started) much earlier.
    xT = din("xT", [D, S])
    wkvr = din("wkvr", [128, 1024])    # 8 col-blocks: [Wv_g | Wk_g] k-tiles
    wqr = din("wqr", [128, 2048])      # 8 col-blocks: Wq k-tiles (256 wide)
    qtab = din("qtab", [128, 4 * S])   # [cq0 | cq1 | sq0 | sq1]
    cksk = din("cksk", [HD, 2 * S])    # [ck | sk]
    wor = din("wor", [128, 2048])      # [Wo rows 0-127 | rows 128-255]
    misc = din("misc", [128, 384])     # [perm | ident | osel]
    outT = nc.dram_tensor("outT", [D, S], bf16, kind="ExternalOutput").ap()

    with tile.TileContext(nc) as tc, ExitStack() as ctx:
        # ---------------- persistent SBUF tensors ----------------
        pers = ctx.enter_context(tc.tile_pool(name="pers", bufs=1))
        xt_s = [pers.tile([128, S], bf16, tag=f"xt{i}", name=f"xt{i}") for i in range(8)]
        wkv_a = pers.tile([128, 1024], bf16, tag="wkva", name="wkva")
        wq_a = pers.tile([128, 2048], bf16, tag="wqa", name="wqa")
        qtab_s = pers.tile([128, 4 * S], bf16, tag="qtab", name="qtab")
        ktab_s = pers.tile([128, 2 * S], bf16, tag="ktab", name="ktab")
        wo_a = pers.tile([128, 2048], bf16, tag="woa", name="woa")
        misc_s = pers.tile([128, 384], bf16, tag="misc", name="misc")
        # persistent denominator staging tiles for the row-selector
        # broadcast (rows other than 0/32 stay zero forever)
        denrp = [pers.tile([128, QB], bf16, tag=f"dn{i}", name=f"dn{i}")
                 for i in range(2)]

        # zero-padded per-head q tiles: qpad[pr][hx] has head (2pr+hx)'s
        # rotated q in rows 64*hx..64*hx+64 and ZEROS elsewhere, so scores
        # run as plain K=128 matmuls against kdup=[k;k] (no PE mode switch
        # between scores and PV -> no TensorE drains in the inner loop).
        qpad = [[pers.tile([128, S], bf16, tag=f"qp{i}{j}", name=f"qp{i}{j}")
                 for j in range(2)] for i in range(2)]
        kv_sb = pers.tile([128, S], bf16, tag="kv", name="kv")     # v rows 0-63, k rows 64-127
        kdup = pers.tile([128, S], bf16, tag="kdup", name="kdup")
        v_t = [pers.tile([128, HD + 1], bf16, tag=f"v{i}", name=f"v{i}") for i in range(NKT)]
        ctxn2 = [pers.tile([128, S], bf16, tag=f"cx{i}", name=f"cx{i}") for i in range(2)]

        # DMA order: weights + misc first (small), then the x tiles stream
        # in so pass 1 consumes them as they arrive; rope tables next; wo
        # last (phase D only).
        nc.sync.dma_start(wkv_a[:], wkvr[:])
        nc.sync.dma_start(wq_a[:], wqr[:])
        nc.sync.dma_start(misc_s[:], misc[:])
        for i in range(8):
            nc.sync.dma_start(xt_s[i][:], xT[128 * i:128 * (i + 1), :])
        nc.sync.dma_start(ktab_s[64:128, :], cksk[:])
        nc.sync.dma_start(qtab_s[:], qtab[:])
        nc.sync.dma_start(wo_a[:], wor[:])
        for i in range(2):
            nc.vector.memset(qpad[i][0][64:128, :], 0.0)
            nc.vector.memset(qpad[i][1][0:64, :], 0.0)
            nc.vector.memset(denrp[i][:], 0.0)

        # ---------------- phase B: projections + rope ----------------
        with tc.tile_pool(name="pj_ps", bufs=4, space="PSUM") as pj_ps, \
             tc.tile_pool(name="pj_swp", bufs=2, space="PSUM") as pj_swp, \
             tc.tile_pool(name="pj_aux", bufs=2, space="PSUM") as pj_aux, \
             tc.tile_pool(name="pj_sb", bufs=4) as pj_sb:

            def q_rope(mc, qc, q_ps_tile):
                sl = slice(qc * QC, (qc + 1) * QC)
                raw = pj_sb.tile([128, QC], bf16, tag="qraw", name="qraw")
                nc.vector.tensor_copy(raw[:], q_ps_tile[:])
                swp = pj_swp.tile([128, QC], f32, tag="swp", name="swp")
                nc.tensor.matmul(swp[:], misc_s[:, 0:128], raw[:],
                                 start=True, stop=True)
                t1 = pj_sb.tile([128, QC], bf16, tag="t1", name="t1")
                nc.vector.tensor_mul(
                    t1[:], raw[:],
                    qtab_s[:, S * mc + qc * QC:S * mc + (qc + 1) * QC])
                t2 = pj_sb.tile([128, QC], bf16, tag="t2", name="t2")
                nc.vector.tensor_mul(
                    t2[:], swp[:],
                    qtab_s[:, 2 * S + S * mc + qc * QC:
                           2 * S + S * mc + (qc + 1) * QC])
                nc.vector.tensor_add(qpad[mc][0][0:64, sl],
                                     t1[0:64, :], t2[0:64, :])
                nc.vector.tensor_add(qpad[mc][1][64:128, sl],
                                     t1[64:128, :], t2[64:128, :])

            def kv_rope(qc, kv_ps_tile):
                sl = slice(qc * QC, (qc + 1) * QC)
                nc.vector.tensor_copy(kv_sb[:, sl], kv_ps_tile[:])
                # k rope: raw k rows at partitions 64-127
                swp = pj_swp.tile([128, QC], f32, tag="swp", name="swp")
                nc.tensor.matmul(swp[64:128, :], misc_s[64:128, 64:128],
                                 kv_sb[64:128, sl], start=True, stop=True)
                t1 = pj_sb.tile([128, QC], bf16, tag="t1", name="t1")
                nc.vector.tensor_mul(t1[64:128, :], kv_sb[64:128, sl],
                                     ktab_s[64:128, sl])
                t2 = pj_sb.tile([128, QC], bf16, tag="t2", name="t2")
                nc.vector.tensor_mul(
                    t2[64:128, :], swp[64:128, :],
                    ktab_s[64:128, S + qc * QC:S + (qc + 1) * QC])
                nc.vector.tensor_add(kdup[64:128, sl], t1[64:128, :],
                                     t2[64:128, :])
                # duplicate k to partitions 0-63 for the T0 row tile
                nc.sync.dma_start(kdup[0:64, sl], kdup[64:128, sl])

            # ---- pass 1: fused V|K projection + Q head-pair 0, qc-halves,
            # MMs interleaved over arriving x tiles
            for half in range(2):
                qcs = (0, 1) if half == 0 else (2, 3)
                kv_ps = {qc: pj_ps.tile([128, QC], f32, tag="ps",
                                        name=f"kvps{qc}") for qc in qcs}
                q0_ps = {qc: pj_ps.tile([128, QC], f32, tag="ps",
                                        name=f"q0ps{qc}") for qc in qcs}
                for kt in range(8):
                    for qc in qcs:
                        nc.tensor.matmul(
                            kv_ps[qc][:], wkv_a[:, 128 * kt:128 * (kt + 1)],
                            xt_s[kt][:, qc * QC:(qc + 1) * QC],
                            start=(kt == 0), stop=(kt == 7))
                        nc.tensor.matmul(
                            q0_ps[qc][:], wq_a[:, 256 * kt:256 * kt + 128],
                            xt_s[kt][:, qc * QC:(qc + 1) * QC],
                            start=(kt == 0), stop=(kt == 7))
                for qc in qcs:
                    kv_rope(qc, kv_ps[qc])
                    q_rope(0, qc, q0_ps[qc])
            # v transposes: v_t[kt] = (v tile)^T with ones column
            for tt in range(NKT):
                tp = pj_aux.tile([128, QC], bf16, tag="aux", name="aux")
                nc.tensor.transpose(tp[:, :HD],
                                    kv_sb[:HD, 128 * tt:128 * (tt + 1)],
                                    misc_s[0:HD, 128:128 + HD])
                nc.vector.tensor_copy(v_t[tt][:, :HD], tp[:, :HD])
                nc.vector.memset(v_t[tt][:, HD:HD + 1], 1.0)

            # ---- pass 2: Q head-pair 1 (x already resident)
            for half in range(2):
                qcs = (0, 1) if half == 0 else (2, 3)
                q1_ps = {qc: pj_ps.tile([128, QC], f32, tag="ps",
                                        name=f"q1ps{qc}") for qc in qcs}
                for kt in range(8):
                    for qc in qcs:
                        nc.tensor.matmul(
                            q1_ps[qc][:], wq_a[:, 256 * kt + 128:256 * kt + 256],
                            xt_s[kt][:, qc * QC:(qc + 1) * QC],
                            start=(kt == 0), stop=(kt == 7))
                for qc in qcs:
                    q_rope(1, qc, q1_ps[qc])

        # ---------------- phase C: attention ----------------
        # Head pair pr = (2pr, 2pr+1): qrope[pr] partitions 0-63 / 64-127.
        # Scores row-tiled: T0 (kdup[0:64]) and T8 (kdup[64:128]) concurrent.
        INVSQ = 1.0 / 32.0  # 1/sqrt(D)
        with tc.tile_pool(name="at_s", bufs=2, space="PSUM") as at_s, \
             tc.tile_pool(name="at_c", bufs=2, space="PSUM") as at_c, \
             tc.tile_pool(name="at_p", bufs=4) as at_p, \
             tc.tile_pool(name="at_u", bufs=4) as at_u:

            def emit_lazy(d):
                """Deferred half of the softmax normalize: broadcast the
                denominator row via a K=128 row-selector matmul (same PE
                mode as everything else), reciprocal, scale, store."""
                pr_, qb_, ctxu2, b2 = d
                qsl = slice(qb_ * QB, (qb_ + 1) * QB)
                for hx in range(2):
                    bc = at_s.tile([128, QB], f32, tag="s", name="bc")
                    for h2 in range(2):
                        csl = slice(512 * h2, 512 * (h2 + 1))
                        nc.tensor.matmul(bc[0:HD, csl],
                                         misc_s[:, 256 + 64 * hx:256 + 64 * hx + HD],
                                         denrp[b2][:, csl],
                                         start=True, stop=True)
                    rcp = at_u.tile([HD, QB], f32, tag="rcp", name="rcp")
                    nc.vector.reciprocal_approx_fast(rcp[:], bc[0:HD, :])
                    if hx == 0:
                        nc.vector.tensor_mul(ctxn2[pr_][0:HD, qsl],
                                             ctxu2[hx][:], rcp[:])
                    else:
                        tmp = at_u.tile([HD, QB], bf16, tag="tmpb",
                                        name="tmpb")
                        nc.vector.tensor_mul(tmp[:], ctxu2[hx][:], rcp[:])
                        nc.sync.dma_start(ctxn2[pr_][HD:128, qsl], tmp[:])

            deferred = None
            blk = 0
            for qb in range(S // QB):
                q0 = qb * QB
                for pr in range(2):
                    ctxA = at_c.tile([HD + 1, QB], f32, tag="ctx", name="ctxA")
                    ctxB = at_c.tile([HD + 1, QB], f32, tag="ctx", name="ctxB")
                    # Software-pipelined: scores(kt)+exp(kt) emitted BEFORE
                    # PV(kt-1) so the PE queue never head-of-line blocks on
                    # an exp wait. Scores are K=128 matmuls against
                    # kdup=[k;k] with zero-padded per-head q: same PE mode
                    # as PV (no drains) and one shared stationary per kt.
                    # Chunk 0 of each head is issued TWICE (idempotent
                    # overwrite): keeps the PE ~100% busy so the HAM clock
                    # gate stays at full rate through the attention phase.
                    ps = [[None, None] for _ in range(NKT)]
                    ctxs = (ctxA, ctxB)
                    for kt in range(NKT + 1):
                        if kt < NKT:
                            ksl = slice(128 * kt, 128 * (kt + 1))
                            st = [at_s.tile([128, QB], f32, tag="s", name="s0"),
                                  at_s.tile([128, QB], f32, tag="s", name="s1")]
                            for hx in range(2):
                                for h2 in ((0, 0, 1) if hx == 0 else (0, 1)):
                                    csl = slice(512 * h2, 512 * (h2 + 1))
                                    qsl = slice(q0 + 512 * h2,
                                                q0 + 512 * (h2 + 1))
                                    nc.tensor.matmul(st[hx][:, csl],
                                                     kdup[:, ksl],
                                                     qpad[pr][hx][:, qsl],
                                                     start=True, stop=True)
                            for hx in range(2):
                                p = at_p.tile([128, QB], bf16, tag="pT",
                                              name=f"p{hx}")
                                nc.scalar.activation(p[:], st[hx][:], EXP,
                                                     scale=INVSQ)
                                ps[kt][hx] = p
                        if kt == 1 and deferred is not None:
                            # lazy half of the PREVIOUS block's normalize,
                            # emitted here so it fills the boundary instead
                            # of stalling it
                            emit_lazy(deferred)
                            deferred = None
                        if kt >= 1:
                            kp = kt - 1
                            for h2 in range(2):
                                csl = slice(512 * h2, 512 * (h2 + 1))
                                for hx in range(2):
                                    nc.tensor.matmul(ctxs[hx][:, csl],
                                                     v_t[kp][:],
                                                     ps[kp][hx][:, csl],
                                                     start=(kp == 0),
                                                     stop=(kp == NKT - 1))
                    # release half of the normalize: get the denominator row
                    # and the 64 ctx rows out of PSUM quickly, freeing both
                    # ctx banks for the next block.
                    b2 = blk % 2
                    ctxu2 = []
                    for hx, cx in ((0, ctxA), (1, ctxB)):
                        nc.scalar.copy(denrp[b2][32 * hx:32 * hx + 1, :],
                                       cx[HD:HD + 1, :])
                        cu = at_u.tile([HD, QB], bf16, tag="ctxu",
                                       name=f"ctxu{hx}")
                        nc.vector.tensor_copy(cu[:], cx[0:HD, :])
                        ctxu2.append(cu)
                    deferred = (pr, qb, ctxu2, b2)
                    blk += 1
            # qb=0 output projection runs here, inside the attention pools
            # (psum from the scores ring): its ctxn2 columns were finalized
            # mid-attention, so these matmuls keep the PE dense while the
            # final block's normalize completes.
            for qc in range(2):
                sl = slice(qc * QC, (qc + 1) * QC)
                for mc in range(8):
                    wp = at_s.tile([128, QB], f32, tag="s", name="wp")
                    for pr in range(2):
                        nc.tensor.matmul(
                            wp[:, 0:QC],
                            wo_a[:, 1024 * pr + 128 * mc:
                                 1024 * pr + 128 * (mc + 1)],
                            ctxn2[pr][:, sl], start=(pr == 0), stop=(pr == 1))
                    ob = at_u.tile([128, QC], bf16, tag="ob2", name="ob2")
                    if mc % 2 == 0:
                        nc.vector.tensor_copy(ob[:], wp[:, 0:QC])
                    else:
                        nc.scalar.copy(ob[:], wp[:, 0:QC])
                    nc.sync.dma_start(outT[128 * mc:128 * (mc + 1), sl], ob[:])
            emit_lazy(deferred)

        # ---------------- phase D: output projection (qb=1 columns) ------
        with tc.tile_pool(name="wo_ps", bufs=8, space="PSUM") as wo_ps, \
             tc.tile_pool(name="wo_sb", bufs=6) as wo_sb:
            for qc in range(2, NQC):
                sl = slice(qc * QC, (qc + 1) * QC)
                o_ps = [wo_ps.tile([128, QC], f32, tag="wops", name=f"wops{mc}")
                        for mc in range(8)]
                for pr in range(2):
                    for mc in range(8):
                        nc.tensor.matmul(
                            o_ps[mc][:], wo_a[:, 1024 * pr + 128 * mc:1024 * pr + 128 * (mc + 1)],
                            ctxn2[pr][:, sl], start=(pr == 0), stop=(pr == 1))
                for mc in range(8):
                    ob = wo_sb.tile([128, QC], bf16, tag="ob", name="ob")
                    if mc % 2 == 0:
                        nc.vector.tensor_copy(ob[:], o_ps[mc][:])
                    else:
                        nc.scalar.copy(ob[:], o_ps[mc][:])
                    nc.sync.dma_start(outT[128 * mc:128 * (mc + 1), sl], ob[:])

    nc.compile()
    return nc


def _host_inputs(x, Wq, Wk, Wv, Wo):
    """Build the 8 per-core input maps."""
    bf = ml_dtypes.bfloat16
    inv = 1.0 / (THETA ** (np.arange(0, D, 2, dtype=np.float64) / D))
    t = np.arange(S, dtype=np.float64)
    sgn256 = np.where(np.arange(256) % 2 == 0, -1.0, 1.0)
    sgn64 = sgn256[:HD]

    perm = np.zeros((128, 128), np.float32)
    idx = np.arange(128)
    perm[idx ^ 1, idx] = 1.0
    ident = np.eye(128, dtype=np.float32)
    osel = np.zeros((128, 128), np.float32)
    osel[0, 0:64] = 1.0      # head-A denominator row selector
    osel[32, 64:128] = 1.0   # head-B denominator row selector
    misc = np.concatenate([perm, ident, osel], axis=1)  # (128, 384)

    # k rope tables are core-independent
    angk = t[None, :] * inv[np.arange(HD) // 2][:, None]
    ck = np.cos(angk)
    sk = sgn64[:, None] * np.sin(angk)
    cksk = np.concatenate([ck, sk], axis=1).astype(bf)  # (64, 2S)

    in_maps = []
    for c in range(NCORES):
        b, g = divmod(c, G)
        fq = inv[128 * g + np.arange(256) // 2]
        angq = t[None, :] * fq[:, None]
        cq = np.cos(angq)
        sq = sgn256[:, None] * np.sin(angq)
        qtab = np.concatenate(
            [cq[0:128], cq[128:256], sq[0:128], sq[128:256]], axis=1)
        wkv = np.concatenate(
            [Wv[:, HD * g:HD * (g + 1)], Wk[:, HD * g:HD * (g + 1)]], axis=1)
        wkvr = np.concatenate(
            [wkv[128 * k:128 * (k + 1), :] for k in range(8)], axis=1)
        wq_g = Wq[:, 256 * g:256 * (g + 1)]
        wqr = np.concatenate(
            [wq_g[128 * k:128 * (k + 1), :] for k in range(8)], axis=1)
        wo_g = Wo[256 * g:256 * (g + 1), :]
        wor = np.concatenate([wo_g[0:128, :], wo_g[128:256, :]], axis=1)
        in_maps.append({
            "xT": np.ascontiguousarray(x[b].T).astype(bf),
            "wkvr": np.ascontiguousarray(wkvr).astype(bf),
            "wqr": np.ascontiguousarray(wqr).astype(bf),
            "qtab": np.ascontiguousarray(qtab).astype(bf),
            "cksk": cksk,
            "wor": np.ascontiguousarray(wor).astype(bf),
            "misc": misc.astype(bf),
        })
    return in_maps


def _run(in_maps, trace=False, tmpdir=None):
    global _compiled
    from concourse.bass_utils import run_bass_kernel_spmd
    if _compiled is None:
        _compiled = _build_program()
    return run_bass_kernel_spmd(_compiled, in_maps, list(range(NCORES)),
                                trace=trace, tmpdir=tmpdir)


def kernel(x, Wq, Wk, Wv, Wo, _trace=False, _tmpdir=None):
    x = np.asarray(x, np.float32)
    in_maps = _host_inputs(x, np.asarray(Wq, np.float32),
                           np.asarray(Wk, np.float32),
                           np.asarray(Wv, np.float32),
                           np.asarray(Wo, np.float32))
    res = _run(in_maps, trace=_trace, tmpdir=_tmpdir)
    out = np.zeros((B, S, D), np.float32)
    for c in range(NCORES):
        b = c // G
        out[b] += res.results[c]["outT"].T.astype(np.float32)
    kernel.last_results = res
    return out
